# revision 7
# baseline (speedup 1.0000x reference)
"""Single-head causal attention (B=8, T=2048, C=768, H=64) on 8 TRN2 NeuronCores.

Sharding: data-parallel over the batch dim — one batch element per core.

Per-core algorithm (all matmuls contract on the partition dim, fp32r):
  - inputs are fed transposed from the host: xT [C, T], so QKV projections use
    W chunks as the stationary operand and xT as the moving operand.
  - qkT [128, T]: rows 0:64 = q^T, 64:128 = k^T (one fused matmul per chunk
    with lhsT = [Wq | Wk] [128, 128]).
  - vT [64, T], then PE-transposed into natural-layout v tiles [128, 65] with a
    ones column appended.
  - attention works in S^T layout (keys j on partitions, queries i on free):
    S^T(j-chunk, i-range) = kT_j.T @ qT. Scale+exp fused on ScalarE
    (PSUM -> SBUF). Causal: only j <= i blocks are computed; the leading
    128-col diagonal block gets an upper-triangular 0/1 mask multiply.
  - AV: out^T [65, 512-group] += [v_j | 1].T @ expS^T_j; row 64 accumulates the
    softmax denominators for free. Final: out^T[:64] * (1/row64), DMA'd out as
    outT [64, T]; host transposes back.

No max-subtraction in softmax: scores * C**-0.5 are bounded (|s| < ~3), exp is
safe in fp32, and the result is mathematically identical to jax.nn.softmax.
"""

import numpy as np

import concourse.bass as bass
import concourse.tile as tile
from concourse import bacc, mybir
from concourse.bass import ds, ts
from concourse.masks import make_identity, make_upper_triangular

B, T, C, H = 8, 2048, 768, 64
P = 128
NCH = C // P          # 6 contraction chunks for QKV
GW = 512              # output column-group width
NG = T // GW          # 4 groups
NT = T // P           # 16 t-chunks
JPG = GW // P         # 4 j-chunks per group
SCALE = float(C) ** -0.5

F32 = mybir.dt.float32
F32R = mybir.dt.float32r
EXP = mybir.ActivationFunctionType.Exp


def _emit(tc: tile.TileContext, ctx, xT, wqk, wv, outT):
    nc = tc.nc

    consts = ctx.enter_context(tc.tile_pool(name="consts", bufs=1))
    xpool = ctx.enter_context(tc.tile_pool(name="x", bufs=1))
    qpool = ctx.enter_context(tc.tile_pool(name="qkv", bufs=1))

    ident = consts.tile([H, H], F32)
    make_identity(nc, ident[:])
    tri = consts.tile([P, P], F32)
    make_upper_triangular(nc, tri[:], val=1.0, diag=True)

    w_qk = consts.tile([P, NCH, P], F32R)
    nc.sync.dma_start(w_qk[:], wqk.rearrange("(o p) m -> p o m", p=P))
    w_v = consts.tile([P, NCH, H], F32R)
    nc.sync.dma_start(w_v[:], wv.rearrange("(o p) m -> p o m", p=P))

    xT_sb = xpool.tile([P, NCH, T], F32R)
    xTr = xT.rearrange("(o p) t -> p o t", p=P)
    for c in range(NCH):
        for h in range(2):
            sl = ds(h * (T // 2), T // 2)
            nc.sync.dma_start(xT_sb[:, c, sl], xTr[:, c, sl])

    qkT = qpool.tile([P, T], F32R)
    kT = qpool.tile([H, T], F32R)
    vT = qpool.tile([H, T], F32)
    v_sb = qpool.tile([P, NT, H + 1], F32R)

    # ---- Phase 1: QKV projections + v transposes ----
    with (
        tc.tile_pool(name="p1psum", bufs=2, space="PSUM") as pp1,
        tc.tile_pool(name="tpsum", bufs=2, space="PSUM") as tp,
    ):
        for g in range(NG):
            ps = pp1.tile([P, GW], F32, tag="qk")
            for c in range(NCH):
                nc.tensor.matmul(
                    ps[:],
                    w_qk[:, c, :],
                    xT_sb[:, c, ts(g, GW)],
                    start=(c == 0),
                    stop=(c == NCH - 1),
                )
            nc.scalar.copy(qkT[:, ts(g, GW)], ps[:])
            # k^T lives at partitions 64:128; matmul needs both operands at the
            # same base partition, so shift it down via SBUF->SBUF DMA.
            nc.sync.dma_start(kT[:, ts(g, GW)], qkT[H:P, ts(g, GW)])

            ps2 = pp1.tile([H, GW], F32, tag="v")
            for c in range(NCH):
                nc.tensor.matmul(
                    ps2[:],
                    w_v[:, c, :],
                    xT_sb[:, c, ts(g, GW)],
                    start=(c == 0),
                    stop=(c == NCH - 1),
                )
            nc.vector.tensor_copy(vT[:, ts(g, GW)], ps2[:])

        for t in range(NT):
            pt = tp.tile([P, H], F32, tag="vt")
            nc.tensor.transpose(pt[:], vT[:, ts(t, P)], ident[:])
            nc.vector.tensor_copy(v_sb[:, t, 0:H], pt[:])
        nc.vector.memset(v_sb[:, :, H : H + 1].bitcast(F32), 1.0)

    # ---- Phase 2/3: attention ----
    sp = ctx.enter_context(tc.tile_pool(name="spsum", bufs=3, space="PSUM"))
    op = ctx.enter_context(tc.tile_pool(name="opsum", bufs=2, space="PSUM"))
    pb = ctx.enter_context(tc.tile_pool(name="probs", bufs=4))
    fin = ctx.enter_context(tc.tile_pool(name="fin", bufs=2))

    def emit_probs(g, jj):
        istart = max(g * GW, jj * P)
        n = (g + 1) * GW - istart
        sps = sp.tile([P, GW], F32, tag="s")
        nc.tensor.matmul(
            sps[:, :n],
            kT[:, ts(jj, P)],
            qkT[0:H, ds(istart, n)],
            start=True,
            stop=True,
        )
        prb = pb.tile([P, GW], F32R, tag="p")
        nc.scalar.activation(prb[:, :n], sps[:, :n], EXP, scale=SCALE)
        if jj >= JPG * g:
            # leading 128 cols are the diagonal block: upper-tri (j<=i) mask
            nc.vector.tensor_mul(out=prb[:, :P], in0=prb[:, :P], in1=tri[:])
        return prb

    pairs = [(g, jj) for g in range(NG) for jj in range(JPG * g + JPG)]
    ops_by_g = {}
    prb_next = emit_probs(*pairs[0])
    for idx, (g, jj) in enumerate(pairs):
        prb = prb_next
        if idx + 1 < len(pairs):
            prb_next = emit_probs(*pairs[idx + 1])

        if jj == 0:
            ops_by_g[g] = op.tile([H + 1, GW], F32, tag="o", name=f"ops_{g}")
        ops = ops_by_g[g]
        istart = max(g * GW, jj * P)
        n = (g + 1) * GW - istart
        ioff = istart - g * GW
        nc.tensor.matmul(
            ops[:, ioff : ioff + n],
            v_sb[:, jj, :],
            prb[:, :n],
            start=(jj == 0),
            stop=(jj == JPG * g + JPG - 1),
        )

        if jj == JPG * g + JPG - 1:
            rec = fin.tile([1, GW], F32, tag="rec")
            nc.vector.reciprocal(rec[:], ops[H : H + 1, :])
            recb = fin.tile([H, GW], F32, tag="recb")
            nc.gpsimd.partition_broadcast(recb[:], rec[:])
            o_sb = fin.tile([H, GW], F32, tag="osb")
            nc.vector.tensor_mul(out=o_sb[:], in0=ops[0:H, :], in1=recb[:])
            nc.sync.dma_start(outT[:, ts(g, GW)], o_sb[:])


def build():
    from contextlib import ExitStack

    nc = bacc.Bacc("TRN2", target_bir_lowering=False, debug=False, num_devices=B)
    xT = nc.dram_tensor("xT", [C, T], F32R, kind="ExternalInput").ap()
    wqk = nc.dram_tensor("wqk", [C, P], F32R, kind="ExternalInput").ap()
    wv = nc.dram_tensor("wv", [C, H], F32R, kind="ExternalInput").ap()
    outT = nc.dram_tensor("outT", [H, T], F32, kind="ExternalOutput").ap()
    with tile.TileContext(nc) as tc, ExitStack() as ctx:
        _emit(tc, ctx, xT, wqk, wv, outT)
    nc.compile()
    return nc


_NC = None


def _get_nc():
    global _NC
    if _NC is None:
        _NC = build()
    return _NC


def make_in_maps(x, Wk, Wq, Wv):
    wqk = np.ascontiguousarray(np.concatenate([Wq, Wk], axis=1), dtype=np.float32)
    wv = np.ascontiguousarray(Wv, dtype=np.float32)
    return [
        {
            "xT": np.ascontiguousarray(np.asarray(x[b]).T, dtype=np.float32),
            "wqk": wqk,
            "wv": wv,
        }
        for b in range(B)
    ]


def kernel(x, Wk, Wq, Wv):
    from concourse.bass_utils import run_bass_kernel_spmd

    nc = _get_nc()
    in_maps = make_in_maps(x, Wk, Wq, Wv)
    r = run_bass_kernel_spmd(nc, in_maps, core_ids=list(range(B)))
    out = np.stack([r.results[b]["outT"].T for b in range(B)])
    return np.ascontiguousarray(out, dtype=np.float32)


# revision 10
# speedup vs baseline: 1.1891x; 1.1891x over previous
"""Single-head causal attention (B=8, T=2048, C=768, H=64) on 8 TRN2 NeuronCores.

Sharding: data-parallel over the batch dim — one batch element per core.

Per-core algorithm (all matmuls contract on the partition dim, bf16 operands,
fp32 PSUM accumulation):
  - inputs are fed transposed and pre-cast to bf16 from the host: xT [C, T].
  - qkT [128, T]: rows 0:64 = q^T, 64:128 = k^T (one fused matmul per chunk
    with lhsT = [Wq | Wk] [128, 128]); the k^T half is shifted to a base-0
    tile via SBUF->SBUF DMA (matmul operands must share a base partition).
  - vT [64, T], then PE-transposed into natural-layout v tiles [128, 65] with a
    ones column appended.
  - attention works in S^T layout (keys j on partitions, queries i on free):
    S^T(j-chunk, i-range) = kT_j.T @ qT. Scale+exp fused on ScalarE
    (PSUM -> SBUF, bf16 out). Causal: only j <= i blocks are computed; the
    leading 128-col diagonal block gets an upper-triangular 0/1 mask multiply.
  - AV: out^T [65, 1024-group] += [v_j | 1].T @ expS^T_j; row 64 accumulates
    the softmax denominators for free. Final: out^T[:64] * (1/row64) in fp32
    (reciprocal runs on a [64, 16] DMA-reshaped view of the sums — a [1, N]
    reciprocal is serial on one DVE lane and ~17x slower). Output outT [64, T]
    fp32; host transposes back.

No max-subtraction in softmax: scores * C**-0.5 are bounded (|s| < ~3), exp is
safe in fp32, and the result is mathematically identical to jax.nn.softmax.
"""

import ml_dtypes
import numpy as np

import concourse.bass as bass
import concourse.tile as tile
from concourse import bacc, mybir
from concourse.bass import ds, ts
from concourse.masks import make_identity, make_upper_triangular

B, T, C, H = 8, 2048, 768, 64
P = 128
NCH = C // P          # 6 contraction chunks for QKV
GW = 1024             # attention output column-group width
NG = T // GW          # 2 groups
NT = T // P           # 16 t-chunks
JPG = GW // P         # 8 j-chunks per group
SCALE = float(C) ** -0.5

F32 = mybir.dt.float32
BF16 = mybir.dt.bfloat16
EXP = mybir.ActivationFunctionType.Exp


def _emit(tc: tile.TileContext, ctx, xT, wqk, wv, outT):
    nc = tc.nc

    consts = ctx.enter_context(tc.tile_pool(name="consts", bufs=1))
    xpool = ctx.enter_context(tc.tile_pool(name="x", bufs=1))
    qpool = ctx.enter_context(tc.tile_pool(name="qkv", bufs=1))

    ident = consts.tile([H, H], BF16)
    make_identity(nc, ident[:])
    tri = consts.tile([P, P], BF16)
    make_upper_triangular(nc, tri[:], val=1.0, diag=True)

    w_qk = consts.tile([P, NCH, P], BF16)
    nc.sync.dma_start(w_qk[:], wqk.rearrange("(o p) m -> p o m", p=P))
    w_v = consts.tile([P, NCH, H], BF16)
    nc.sync.dma_start(w_v[:], wv.rearrange("(o p) m -> p o m", p=P))

    xT_sb = xpool.tile([P, NCH, T], BF16)
    xTr = xT.rearrange("(o p) t -> p o t", p=P)
    for c in range(NCH):
        # split across the two HWDGE queues (SP + ACT) for DMA parallelism
        eng = nc.sync if c % 2 == 0 else nc.scalar
        eng.dma_start(xT_sb[:, c, :], xTr[:, c, :])

    qkT = qpool.tile([P, T], BF16)
    kT = qpool.tile([H, T], BF16)
    vT = qpool.tile([H, T], BF16)
    v_sb = qpool.tile([P, NT, H + 1], BF16)

    # ---- Phase 1: QKV projections + v transposes ----
    with (
        tc.tile_pool(name="p1psum", bufs=2, space="PSUM") as pp1,
        tc.tile_pool(name="tpsum", bufs=2, space="PSUM") as tp,
    ):
        for g in range(4):
            ps = pp1.tile([P, 512], F32, tag="qk")
            for c in range(NCH):
                nc.tensor.matmul(
                    ps[:],
                    w_qk[:, c, :],
                    xT_sb[:, c, ts(g, 512)],
                    start=(c == 0),
                    stop=(c == NCH - 1),
                )
            nc.vector.tensor_copy(qkT[:, ts(g, 512)], ps[:])
            # k^T lives at partitions 64:128; matmul needs both operands at the
            # same base partition, so shift it down via SBUF->SBUF DMA.
            nc.sync.dma_start(kT[:, ts(g, 512)], qkT[H:P, ts(g, 512)])

            ps2 = pp1.tile([H, 512], F32, tag="v")
            for c in range(NCH):
                nc.tensor.matmul(
                    ps2[:],
                    w_v[:, c, :],
                    xT_sb[:, c, ts(g, 512)],
                    start=(c == 0),
                    stop=(c == NCH - 1),
                )
            nc.vector.tensor_copy(vT[:, ts(g, 512)], ps2[:])

        for t in range(NT):
            pt = tp.tile([P, H], BF16, tag="vt")
            nc.tensor.transpose(pt[:], vT[:, ts(t, P)], ident[:])
            nc.vector.tensor_copy(v_sb[:, t, 0:H], pt[:])
        nc.vector.memset(v_sb[:, :, H : H + 1], 1.0)

    # ---- Phase 2/3: attention ----
    sp = ctx.enter_context(tc.tile_pool(name="spsum", bufs=2, space="PSUM"))
    op = ctx.enter_context(tc.tile_pool(name="opsum", bufs=2, space="PSUM"))
    pb = ctx.enter_context(tc.tile_pool(name="probs", bufs=3))
    fin = ctx.enter_context(tc.tile_pool(name="fin", bufs=2))

    def emit_probs(g, jj):
        istart = max(g * GW, jj * P)
        n = (g + 1) * GW - istart
        sps = sp.tile([P, GW], F32, tag="s")
        for h in range(0, n, 512):
            nh = min(512, n - h)
            nc.tensor.matmul(
                sps[:, h : h + nh],
                kT[:, ts(jj, P)],
                qkT[0:H, ds(istart + h, nh)],
                start=True,
                stop=True,
            )
        prb = pb.tile([P, GW], BF16, tag="p")
        nc.scalar.activation(prb[:, :n], sps[:, :n], EXP, scale=SCALE)
        if jj >= JPG * g:
            # leading 128 cols are the diagonal block: upper-tri (j<=i) mask
            nc.vector.tensor_mul(out=prb[:, :P], in0=prb[:, :P], in1=tri[:])
        return prb

    pairs = [(g, jj) for g in range(NG) for jj in range(JPG * g + JPG)]
    ops_by_g = {}
    prb_next = emit_probs(*pairs[0])
    for idx, (g, jj) in enumerate(pairs):
        prb = prb_next
        if idx + 1 < len(pairs):
            prb_next = emit_probs(*pairs[idx + 1])

        if jj == 0:
            ops_by_g[g] = op.tile([H + 1, GW], F32, tag="o", name=f"ops_{g}")
        ops = ops_by_g[g]
        istart = max(g * GW, jj * P)
        n = (g + 1) * GW - istart
        ioff = istart - g * GW
        # split at the ops tile's absolute 512-col PSUM bank boundaries
        seg = ioff
        while seg < ioff + n:
            seg_end = min(ioff + n, (seg // 512 + 1) * 512)
            half = seg // 512
            # last j-chunk writing this 512-wide half of the group
            jj_last = min(JPG * g + JPG - 1, JPG * g + 4 * (half + 1) - 1)
            nc.tensor.matmul(
                ops[:, seg:seg_end],
                v_sb[:, jj, :],
                prb[:, seg - ioff : seg_end - ioff],
                start=(jj == 0),
                stop=(jj == jj_last),
            )
            seg = seg_end

        if jj == JPG * g + JPG - 1:
            svec = fin.tile([1, GW], F32, tag="svec")
            nc.vector.tensor_copy(svec[:], ops[H : H + 1, :])
            srect = fin.tile([H, GW // H], F32, tag="srect")
            nc.sync.dma_start(srect[:], svec[:])
            srecr = fin.tile([H, GW // H], F32, tag="srecr")
            nc.vector.reciprocal(srecr[:], srect[:])
            rvec = fin.tile([1, GW], F32, tag="rvec")
            nc.sync.dma_start(rvec[:], srecr[:])
            recb = fin.tile([H, GW], F32, tag="recb")
            nc.gpsimd.partition_broadcast(recb[:], rvec[:])
            o_sb = fin.tile([H, GW], F32, tag="osb")
            nc.vector.tensor_mul(out=o_sb[:], in0=ops[0:H, :], in1=recb[:])
            nc.sync.dma_start(outT[:, ts(g, GW)], o_sb[:])


def build():
    from contextlib import ExitStack

    nc = bacc.Bacc("TRN2", target_bir_lowering=False, debug=False, num_devices=B)
    xT = nc.dram_tensor("xT", [C, T], BF16, kind="ExternalInput").ap()
    wqk = nc.dram_tensor("wqk", [C, P], BF16, kind="ExternalInput").ap()
    wv = nc.dram_tensor("wv", [C, H], BF16, kind="ExternalInput").ap()
    outT = nc.dram_tensor("outT", [H, T], F32, kind="ExternalOutput").ap()
    with tile.TileContext(nc) as tc, ExitStack() as ctx:
        _emit(tc, ctx, xT, wqk, wv, outT)
    nc.compile()
    return nc


_NC = None


def _get_nc():
    global _NC
    if _NC is None:
        _NC = build()
    return _NC


def make_in_maps(x, Wk, Wq, Wv):
    bf = ml_dtypes.bfloat16
    wqk = np.ascontiguousarray(np.concatenate([Wq, Wk], axis=1)).astype(bf)
    wv = np.ascontiguousarray(np.asarray(Wv)).astype(bf)
    return [
        {
            "xT": np.ascontiguousarray(np.asarray(x[b]).T).astype(bf),
            "wqk": wqk,
            "wv": wv,
        }
        for b in range(B)
    ]


def kernel(x, Wk, Wq, Wv):
    from concourse.bass_utils import run_bass_kernel_spmd

    nc = _get_nc()
    in_maps = make_in_maps(x, Wk, Wq, Wv)
    r = run_bass_kernel_spmd(nc, in_maps, core_ids=list(range(B)))
    out = np.stack([r.results[b]["outT"].T for b in range(B)])
    return np.ascontiguousarray(out, dtype=np.float32)


# revision 11
# speedup vs baseline: 1.2775x; 1.0743x over previous
"""Single-head causal attention (B=8, T=2048, C=768, H=64) on 8 TRN2 NeuronCores.

Sharding: data-parallel over the batch dim — one batch element per core.

Per-core algorithm (bf16 matmul operands, fp32 PSUM accumulation):
  - inputs fed transposed + pre-cast to bf16 from the host: xT [C, T].
  - warmup: ~14 dummy matmuls at kernel start keep the PE HAM activity monitor
    busy so the array reaches 2.4 GHz before real work arrives.
  - xT DMA'd in column-major halves (all 6 C-chunks of cols [0:1024) first)
    across both HWDGE queues, so QKV matmuls start as early as possible.
  - qkT [128, T]: rows 0:64 = q^T, 64:128 = k^T (fused [Wq | Wk] weights);
    k^T half shifted to a base-0 tile via SBUF->SBUF DMA (matmul operands
    must share a base partition).
  - vT [64, T], PE-transposed (interleaved into phase 1 to avoid PE-idle
    windows) into natural-layout v tiles [128, 65] with a ones column.
  - attention in S^T layout (keys j on partitions, queries i on free):
    S^T(j-chunk, i-range) = kT_j.T @ qT, 1024-wide column groups. Scale+exp
    fused on ScalarE (PSUM -> SBUF bf16). Causal: only j <= i blocks are
    computed; leading 128-col diagonal block gets an upper-tri mask multiply.
  - AV: out^T [65, group] += [v_j | 1].T @ expS^T_j; row 64 accumulates the
    softmax denominators for free.
  - finalize per group: copy out^T to SBUF, PE-transpose [65,128] blocks to
    [128,65] (fp32), then per-partition reciprocal of col 64 and
    tensor_scalar multiply -> natural-layout output o [T, H]. No transposed
    output, no serial [1,N] reciprocal.

No max-subtraction in softmax: scores * C**-0.5 are bounded (|s| < ~3), exp is
safe in fp32, and the result is mathematically identical to jax.nn.softmax.
"""

import ml_dtypes
import numpy as np

import concourse.bass as bass
import concourse.tile as tile
from concourse import bacc, mybir
from concourse.bass import ds, ts
from concourse.masks import make_identity, make_upper_triangular

B, T, C, H = 8, 2048, 768, 64
P = 128
NCH = C // P          # 6 contraction chunks for QKV
GW = 1024             # attention output column-group width
NG = T // GW          # 2 groups
NT = T // P           # 16 t-chunks
JPG = GW // P         # 8 j-chunks per group
SCALE = float(C) ** -0.5
N_WARMUP = 14

F32 = mybir.dt.float32
BF16 = mybir.dt.bfloat16
EXP = mybir.ActivationFunctionType.Exp


def _emit(tc: tile.TileContext, ctx, xT, wqk, wv, out):
    nc = tc.nc

    consts = ctx.enter_context(tc.tile_pool(name="consts", bufs=1))
    xpool = ctx.enter_context(tc.tile_pool(name="x", bufs=1))
    qpool = ctx.enter_context(tc.tile_pool(name="qkv", bufs=1))

    ident = consts.tile([H, H], BF16)
    make_identity(nc, ident[:])
    ident65 = consts.tile([H + 1, H + 1], F32)
    make_identity(nc, ident65[:])
    tri = consts.tile([P, P], BF16)
    make_upper_triangular(nc, tri[:], val=1.0, diag=True)

    w_qk = consts.tile([P, NCH, P], BF16)
    nc.sync.dma_start(w_qk[:], wqk.rearrange("(o p) m -> p o m", p=P))
    w_v = consts.tile([P, NCH, H], BF16)
    nc.scalar.dma_start(w_v[:], wv.rearrange("(o p) m -> p o m", p=P))

    # xT pieces, column-major halves: all chunks' cols [0:1024) land first
    xT_sb = xpool.tile([P, NCH, T], BF16)
    xTr = xT.rearrange("(o p) t -> p o t", p=P)
    pieces = [(h, c) for h in range(2) for c in range(NCH)]
    for i, (h, c) in enumerate(pieces):
        eng = nc.sync if i % 2 == 0 else nc.scalar
        sl = ds(h * (T // 2), T // 2)
        eng.dma_start(xT_sb[:, c, sl], xTr[:, c, sl])

    qkT = qpool.tile([P, T], BF16)
    kT = qpool.tile([H, T], BF16)
    vT = qpool.tile([H, T], BF16)
    v_sb = qpool.tile([P, NT, H + 1], BF16)

    # warmup tile for dummy matmuls
    dum = qpool.tile([P, 512], BF16)
    nc.vector.memset(dum[:], 0.0)

    # ---- Phase 1: QKV projections + v transposes (interleaved) ----
    with (
        tc.tile_pool(name="p1psum", bufs=2, space="PSUM") as pp1,
        tc.tile_pool(name="tpsum", bufs=2, space="PSUM") as tp,
    ):
        for w in range(N_WARMUP):
            dps = pp1.tile([P, 512], F32, tag="qk", name=f"warm_{w}")
            nc.tensor.matmul(dps[:], dum[:, 0:P], dum[:], start=True, stop=True)

        def emit_transposes(g):
            for t in range(4 * g, 4 * g + 4):
                pt = tp.tile([P, H], BF16, tag="vt", name=f"vt_{t}")
                nc.tensor.transpose(pt[:], vT[:, ts(t, P)], ident[:])
                nc.vector.tensor_copy(v_sb[:, t, 0:H], pt[:])

        for g in range(4):
            ps = pp1.tile([P, 512], F32, tag="qk")
            for c in range(NCH):
                nc.tensor.matmul(
                    ps[:],
                    w_qk[:, c, :],
                    xT_sb[:, c, ts(g, 512)],
                    start=(c == 0),
                    stop=(c == NCH - 1),
                )
            nc.vector.tensor_copy(qkT[:, ts(g, 512)], ps[:])
            # k^T lives at partitions 64:128; shift to base partition 0
            nc.scalar.dma_start(kT[:, ts(g, 512)], qkT[H:P, ts(g, 512)])

            ps2 = pp1.tile([H, 512], F32, tag="v")
            for c in range(NCH):
                nc.tensor.matmul(
                    ps2[:],
                    w_v[:, c, :],
                    xT_sb[:, c, ts(g, 512)],
                    start=(c == 0),
                    stop=(c == NCH - 1),
                )
            nc.vector.tensor_copy(vT[:, ts(g, 512)], ps2[:])
            if g >= 1:
                emit_transposes(g - 1)
        emit_transposes(3)
        nc.vector.memset(v_sb[:, :, H : H + 1], 1.0)

    # ---- Phase 2/3: attention ----
    sp = ctx.enter_context(tc.tile_pool(name="spsum", bufs=2, space="PSUM"))
    op = ctx.enter_context(tc.tile_pool(name="opsum", bufs=1, space="PSUM"))
    fp = ctx.enter_context(tc.tile_pool(name="fpsum", bufs=2, space="PSUM"))
    pb = ctx.enter_context(tc.tile_pool(name="probs", bufs=3))
    fin = ctx.enter_context(tc.tile_pool(name="fin", bufs=3))

    def emit_probs(g, jj):
        istart = max(g * GW, jj * P)
        n = (g + 1) * GW - istart
        sps = sp.tile([P, GW], F32, tag="s")
        for h in range(0, n, 512):
            nh = min(512, n - h)
            nc.tensor.matmul(
                sps[:, h : h + nh],
                kT[:, ts(jj, P)],
                qkT[0:H, ds(istart + h, nh)],
                start=True,
                stop=True,
            )
        prb = pb.tile([P, GW], BF16, tag="p")
        nc.scalar.activation(prb[:, :n], sps[:, :n], EXP, scale=SCALE)
        if jj >= JPG * g:
            # leading 128 cols are the diagonal block: upper-tri (j<=i) mask
            nc.vector.tensor_mul(out=prb[:, :P], in0=prb[:, :P], in1=tri[:])
        return prb

    def emit_finalize(g, ops):
        for hh in range(2):
            ot = fin.tile([H + 1, 512], F32, tag="ot", name=f"ot_{g}_{hh}")
            nc.vector.tensor_copy(ot[:], ops[:, ts(hh, 512)])
            for t8 in range(4):
                tt = g * (GW // P) + hh * 4 + t8
                ptf = fp.tile([P, H + 1], F32, tag="ft", name=f"ft_{tt}")
                nc.tensor.transpose(ptf[:], ot[:, ts(t8, P)], ident65[:])
                rch = fin.tile([P, 1], F32, tag="rch", name=f"rch_{tt}")
                nc.vector.reciprocal(rch[:], ptf[:, H : H + 1])
                o_nat = fin.tile([P, H], F32, tag="onat", name=f"onat_{tt}")
                nc.vector.tensor_scalar_mul(o_nat[:], ptf[:, 0:H], rch[:])
                nc.sync.dma_start(out[ts(tt, P), :], o_nat[:])

    pairs = [(g, jj) for g in range(NG) for jj in range(JPG * g + JPG)]
    ops_by_g = {}
    pending_finalize = None
    prb_next = emit_probs(*pairs[0])
    for idx, (g, jj) in enumerate(pairs):
        prb = prb_next
        if idx + 1 < len(pairs):
            prb_next = emit_probs(*pairs[idx + 1])

        if jj == 0:
            ops_by_g[g] = op.tile([H + 1, GW], F32, tag="o", name=f"ops_{g}")
        ops = ops_by_g[g]
        istart = max(g * GW, jj * P)
        n = (g + 1) * GW - istart
        ioff = istart - g * GW
        # split at the ops tile's absolute 512-col PSUM bank boundaries
        seg = ioff
        while seg < ioff + n:
            seg_end = min(ioff + n, (seg // 512 + 1) * 512)
            half = seg // 512
            # last j-chunk writing this 512-wide half of the group
            jj_last = min(JPG * g + JPG - 1, JPG * g + 4 * (half + 1) - 1)
            nc.tensor.matmul(
                ops[:, seg:seg_end],
                v_sb[:, jj, :],
                prb[:, seg - ioff : seg_end - ioff],
                start=(jj == 0),
                stop=(jj == jj_last),
            )
            seg = seg_end

        # delayed by one pair so the finalize copy overlaps the next group's
        # first matmuls instead of stalling the PE
        if pending_finalize is not None:
            emit_finalize(*pending_finalize)
            pending_finalize = None
        if jj == JPG * g + JPG - 1:
            pending_finalize = (g, ops)
    emit_finalize(*pending_finalize)


def build():
    from contextlib import ExitStack

    nc = bacc.Bacc("TRN2", target_bir_lowering=False, debug=False, num_devices=B)
    xT = nc.dram_tensor("xT", [C, T], BF16, kind="ExternalInput").ap()
    wqk = nc.dram_tensor("wqk", [C, P], BF16, kind="ExternalInput").ap()
    wv = nc.dram_tensor("wv", [C, H], BF16, kind="ExternalInput").ap()
    out = nc.dram_tensor("o", [T, H], F32, kind="ExternalOutput").ap()
    with tile.TileContext(nc) as tc, ExitStack() as ctx:
        _emit(tc, ctx, xT, wqk, wv, out)
    nc.compile()
    return nc


_NC = None


def _get_nc():
    global _NC
    if _NC is None:
        _NC = build()
    return _NC


def make_in_maps(x, Wk, Wq, Wv):
    bf = ml_dtypes.bfloat16
    wqk = np.ascontiguousarray(np.concatenate([Wq, Wk], axis=1)).astype(bf)
    wv = np.ascontiguousarray(np.asarray(Wv)).astype(bf)
    return [
        {
            "xT": np.ascontiguousarray(np.asarray(x[b]).T).astype(bf),
            "wqk": wqk,
            "wv": wv,
        }
        for b in range(B)
    ]


def kernel(x, Wk, Wq, Wv):
    from concourse.bass_utils import run_bass_kernel_spmd

    nc = _get_nc()
    in_maps = make_in_maps(x, Wk, Wq, Wv)
    r = run_bass_kernel_spmd(nc, in_maps, core_ids=list(range(B)))
    out = np.stack([r.results[b]["o"] for b in range(B)])
    return np.ascontiguousarray(out, dtype=np.float32)


# revision 13
# speedup vs baseline: 1.5336x; 1.2005x over previous
"""Single-head causal attention (B=8, T=2048, C=768, H=64) on 8 TRN2 NeuronCores.

Sharding: data-parallel over the batch dim — one batch element per core.

Per-core algorithm (bf16 matmul operands, fp32 PSUM accumulation):
  - inputs fed transposed + pre-cast to bf16 from the host: xT [C, T].
  - warmup: ~14 dummy matmuls at kernel start keep the PE HAM activity monitor
    busy so the array reaches 2.4 GHz before real work arrives.
  - xT DMA'd in column-major halves (all 6 C-chunks of cols [0:1024) first)
    across both HWDGE queues, so QKV matmuls start as early as possible.
  - qkT [128, T]: rows 0:64 = q^T, 64:128 = k^T (fused [Wq | Wk] weights);
    k^T half shifted to a base-0 tile via SBUF->SBUF DMA (matmul operands
    must share a base partition).
  - vT [64, T], PE-transposed (interleaved into phase 1 to avoid PE-idle
    windows) into natural-layout v tiles [128, 65] with a ones column.
  - attention in S^T layout (keys j on partitions, queries i on free):
    S^T(j-chunk, i-range) = kT_j.T @ qT, 1024-wide column groups. Scale+exp
    fused on ScalarE (PSUM -> SBUF bf16). Causal: only j <= i blocks are
    computed; leading 128-col diagonal block gets an upper-tri mask multiply.
  - AV: out^T [65, group] += [v_j | 1].T @ expS^T_j; row 64 accumulates the
    softmax denominators for free.
  - finalize per group: copy out^T to SBUF, PE-transpose [65,128] blocks to
    [128,65] (fp32), then per-partition reciprocal of col 64 and
    tensor_scalar multiply -> natural-layout output o [T, H]. No transposed
    output, no serial [1,N] reciprocal.

No max-subtraction in softmax: scores * C**-0.5 are bounded (|s| < ~3), exp is
safe in fp32, and the result is mathematically identical to jax.nn.softmax.
"""

import ml_dtypes
import numpy as np

import concourse.bass as bass
import concourse.tile as tile
from concourse import bacc, mybir
from concourse.bass import ds, ts
from concourse.masks import make_identity, make_upper_triangular

B, T, C, H = 8, 2048, 768, 64
P = 128
NCH = C // P          # 6 contraction chunks for QKV
GW = 1024             # attention output column-group width
NG = T // GW          # 2 groups
NT = T // P           # 16 t-chunks
JPG = GW // P         # 8 j-chunks per group
SCALE = float(C) ** -0.5
N_WARMUP = 14

F32 = mybir.dt.float32
BF16 = mybir.dt.bfloat16
EXP = mybir.ActivationFunctionType.Exp


def _emit(tc: tile.TileContext, ctx, xT, wqk, wv, out):
    nc = tc.nc

    consts = ctx.enter_context(tc.tile_pool(name="consts", bufs=1))
    xpool = ctx.enter_context(tc.tile_pool(name="x", bufs=1))
    qpool = ctx.enter_context(tc.tile_pool(name="qkv", bufs=1))

    ident = consts.tile([H, H], BF16)
    make_identity(nc, ident[:])
    ident65 = consts.tile([H + 1, H + 1], F32)
    make_identity(nc, ident65[:])
    tri = consts.tile([P, P], BF16)
    make_upper_triangular(nc, tri[:], val=1.0, diag=True)

    w_qk = consts.tile([P, NCH, P], BF16)
    nc.sync.dma_start(w_qk[:], wqk.rearrange("(o p) m -> p o m", p=P))
    w_v = consts.tile([P, NCH, H], BF16)
    nc.scalar.dma_start(w_v[:], wv.rearrange("(o p) m -> p o m", p=P))

    # xT pieces, column-major halves: all chunks' cols [0:1024) land first
    xT_sb = xpool.tile([P, NCH, T], BF16)
    xTr = xT.rearrange("(o p) t -> p o t", p=P)
    pieces = [(h, c) for h in range(2) for c in range(NCH)]
    for i, (h, c) in enumerate(pieces):
        eng = nc.sync if i % 2 == 0 else nc.scalar
        sl = ds(h * (T // 2), T // 2)
        eng.dma_start(xT_sb[:, c, sl], xTr[:, c, sl])

    qkT = qpool.tile([P, T], BF16)
    kT = qpool.tile([H, T], BF16)
    vT = qpool.tile([H, T], BF16)
    v_sb = qpool.tile([P, NT, H + 1], BF16)

    # warmup tile for dummy matmuls
    dum = qpool.tile([P, 512], BF16)
    nc.vector.memset(dum[:], 0.0)

    # ---- Phase 1: QKV projections + v transposes (interleaved) ----
    with (
        tc.tile_pool(name="p1psum", bufs=2, space="PSUM") as pp1,
        tc.tile_pool(name="tpsum", bufs=2, space="PSUM") as tp,
    ):
        for w in range(N_WARMUP):
            dps = pp1.tile([P, 512], F32, tag="qk", name=f"warm_{w}")
            nc.tensor.matmul(dps[:], dum[:, 0:P], dum[:], start=True, stop=True)

        def emit_transposes(g):
            for t in range(4 * g, 4 * g + 4):
                pt = tp.tile([P, H], BF16, tag="vt", name=f"vt_{t}")
                nc.tensor.transpose(pt[:], vT[:, ts(t, P)], ident[:])
                nc.vector.tensor_copy(v_sb[:, t, 0:H], pt[:])

        for g in range(4):
            ps = pp1.tile([P, 512], F32, tag="qk")
            for c in range(NCH):
                nc.tensor.matmul(
                    ps[:],
                    w_qk[:, c, :],
                    xT_sb[:, c, ts(g, 512)],
                    start=(c == 0),
                    stop=(c == NCH - 1),
                )
            nc.vector.tensor_copy(qkT[:, ts(g, 512)], ps[:])
            # k^T lives at partitions 64:128; shift to base partition 0
            nc.scalar.dma_start(kT[:, ts(g, 512)], qkT[H:P, ts(g, 512)])

            ps2 = pp1.tile([H, 512], F32, tag="v")
            for c in range(NCH):
                nc.tensor.matmul(
                    ps2[:],
                    w_v[:, c, :],
                    xT_sb[:, c, ts(g, 512)],
                    start=(c == 0),
                    stop=(c == NCH - 1),
                )
            nc.vector.tensor_copy(vT[:, ts(g, 512)], ps2[:])
            if g >= 1:
                emit_transposes(g - 1)
        emit_transposes(3)
        nc.vector.memset(v_sb[:, :, H : H + 1], 1.0)

    # ---- Phase 2/3: attention ----
    sp = ctx.enter_context(tc.tile_pool(name="spsum", bufs=2, space="PSUM"))
    op = ctx.enter_context(tc.tile_pool(name="opsum", bufs=1, space="PSUM"))
    fp = ctx.enter_context(tc.tile_pool(name="fpsum", bufs=2, space="PSUM"))
    pb = ctx.enter_context(tc.tile_pool(name="probs", bufs=6))
    fin = ctx.enter_context(tc.tile_pool(name="fin", bufs=3))

    def emit_probs(g, jj):
        istart = max(g * GW, jj * P)
        n = (g + 1) * GW - istart
        sps = sp.tile([P, GW], F32, tag="s")
        for h in range(0, n, 512):
            nh = min(512, n - h)
            nc.tensor.matmul(
                sps[:, h : h + nh],
                kT[:, ts(jj, P)],
                qkT[0:H, ds(istart + h, nh)],
                start=True,
                stop=True,
            )
        prb = pb.tile([P, GW], BF16, tag="p")
        nc.scalar.activation(prb[:, :n], sps[:, :n], EXP, scale=SCALE)
        if jj >= JPG * g:
            # leading 128 cols are the diagonal block: upper-tri (j<=i) mask
            nc.vector.tensor_mul(out=prb[:, :P], in0=prb[:, :P], in1=tri[:])
        return prb

    def emit_finalize(g, ops):
        for hh in range(2):
            ot = fin.tile([H + 1, 512], F32, tag="ot", name=f"ot_{g}_{hh}")
            nc.vector.tensor_copy(ot[:], ops[:, ts(hh, 512)])
            for t8 in range(4):
                tt = g * (GW // P) + hh * 4 + t8
                ptf = fp.tile([P, H + 1], F32, tag="ft", name=f"ft_{tt}")
                nc.tensor.transpose(ptf[:], ot[:, ts(t8, P)], ident65[:])
                rch = fin.tile([P, 1], F32, tag="rch", name=f"rch_{tt}")
                nc.vector.reciprocal(rch[:], ptf[:, H : H + 1])
                o_nat = fin.tile([P, H], F32, tag="onat", name=f"onat_{tt}")
                nc.vector.tensor_scalar_mul(o_nat[:], ptf[:, 0:H], rch[:])
                nc.sync.dma_start(out[ts(tt, P), :], o_nat[:])

    pairs = [(g, jj) for g in range(NG) for jj in range(JPG * g + JPG)]
    ops_by_g = {}
    pending_finalize = None
    LOOKAHEAD = 2
    prb_queue = [emit_probs(*pairs[i]) for i in range(LOOKAHEAD)]
    for idx, (g, jj) in enumerate(pairs):
        prb = prb_queue.pop(0)
        if idx + LOOKAHEAD < len(pairs):
            prb_queue.append(emit_probs(*pairs[idx + LOOKAHEAD]))

        if jj == 0:
            ops_by_g[g] = op.tile([H + 1, GW], F32, tag="o", name=f"ops_{g}")
        ops = ops_by_g[g]
        istart = max(g * GW, jj * P)
        n = (g + 1) * GW - istart
        ioff = istart - g * GW
        # split at the ops tile's absolute 512-col PSUM bank boundaries
        seg = ioff
        while seg < ioff + n:
            seg_end = min(ioff + n, (seg // 512 + 1) * 512)
            half = seg // 512
            # last j-chunk writing this 512-wide half of the group
            jj_last = min(JPG * g + JPG - 1, JPG * g + 4 * (half + 1) - 1)
            nc.tensor.matmul(
                ops[:, seg:seg_end],
                v_sb[:, jj, :],
                prb[:, seg - ioff : seg_end - ioff],
                start=(jj == 0),
                stop=(jj == jj_last),
            )
            seg = seg_end

        # delayed by one pair so the finalize copy overlaps the next group's
        # first matmuls instead of stalling the PE
        if pending_finalize is not None:
            emit_finalize(*pending_finalize)
            pending_finalize = None
        if jj == JPG * g + JPG - 1:
            pending_finalize = (g, ops)
    emit_finalize(*pending_finalize)


def build():
    from contextlib import ExitStack

    nc = bacc.Bacc("TRN2", target_bir_lowering=False, debug=False, num_devices=B)
    xT = nc.dram_tensor("xT", [C, T], BF16, kind="ExternalInput").ap()
    wqk = nc.dram_tensor("wqk", [C, P], BF16, kind="ExternalInput").ap()
    wv = nc.dram_tensor("wv", [C, H], BF16, kind="ExternalInput").ap()
    out = nc.dram_tensor("o", [T, H], F32, kind="ExternalOutput").ap()
    with tile.TileContext(nc) as tc, ExitStack() as ctx:
        _emit(tc, ctx, xT, wqk, wv, out)
    nc.compile()
    return nc


_NC = None


def _get_nc():
    global _NC
    if _NC is None:
        _NC = build()
    return _NC


def make_in_maps(x, Wk, Wq, Wv):
    bf = ml_dtypes.bfloat16
    wqk = np.ascontiguousarray(np.concatenate([Wq, Wk], axis=1)).astype(bf)
    wv = np.ascontiguousarray(np.asarray(Wv)).astype(bf)
    return [
        {
            "xT": np.ascontiguousarray(np.asarray(x[b]).T).astype(bf),
            "wqk": wqk,
            "wv": wv,
        }
        for b in range(B)
    ]


def kernel(x, Wk, Wq, Wv):
    from concourse.bass_utils import run_bass_kernel_spmd

    nc = _get_nc()
    in_maps = make_in_maps(x, Wk, Wq, Wv)
    r = run_bass_kernel_spmd(nc, in_maps, core_ids=list(range(B)))
    out = np.stack([r.results[b]["o"] for b in range(B)])
    return np.ascontiguousarray(out, dtype=np.float32)
